# revision 66
# baseline (speedup 1.0000x reference)
"""AggregateEmbedding Trainium2 kernel: 8-core SPMD Bass/Tile implementation.

Sharding: data-parallel over cascades/graphs (64 per core). Edges are routed
host-side to the core that owns their destination graph. The 1M-row embedding
table is compacted per core to its touched rows (ids remapped to int16) and
stored bf16 in DRAM; the kernel performs three large transposed dma_gather
calls (cascade / root / leaf), so the random-row gather happens on device with
one ~1us SWDGE fixed cost per call instead of per 128 rows.

Structure on device (per core):
  - xT gathers land as [E=128, n] bf16, directly in matmul-operand layout.
  - Gate pre-activations for the LSTM are accumulated straight into a rotating
    PSUM FIFO: per 2-step block, 12 small matmuls (x @ W_ih, time-onehot @
    TW_ih, pos-onehot @ PW) write the input contribution; at step time 4
    recurrent matmuls (W_hh_k @ h) accumulate on top. No xw SBUF roundtrip.
  - All 4 gates use sigmoid (tanh(z) = 2*sigmoid(2z)-1 with the g-gate weights
    pre-doubled host-side) so one activation call covers the whole gate tile.
  - h history is written to SBUF bf16; the final h per cascade is extracted at
    the end with a selection mask + strided reduce (off the critical path).
  - Tree GNN message passing (both sides) runs interleaved with LSTM steps:
    per 128-edge tile, 2 matmuls (x_src @ W1 + time-onehot @ TW) -> relu on
    DVE -> scatter matmul with a host-built one-hot (scaled by 1/100 for the
    mean) accumulating into a per-side PSUM tile.
  - Final: out^T = relu(sum_k WtT_k^T @ featT_k + b) computed transposed so
    the bias is a per-partition scalar; host transposes back.
"""

import numpy as np
import ml_dtypes

BF16 = ml_dtypes.bfloat16

E = 128
S = 100
B = 512
NCORES = 8
BC = B // NCORES          # cascades / graphs per core
TIME_NUM = 50
MAX_TIME = 1.0
NPG = 100                 # nodes per graph
P = 128
NSAMP = S * BC            # 6400 cascade samples per core
NBLK = NSAMP // P         # 50 two-step blocks

LAST_EXEC_NS = None
TRACE = False
SIMULATE = False            # run TimelineSim cost model after compile
SIM_NS = None
SIM_TRACE_PATH = "/tmp/timeline.pftrace"
SKIP_RUN = False            # skip HW execution (sim-only iteration)

LOOKAHEAD = 3               # gate-FIFO psum blocks in flight
GNN_T0 = 20                 # first LSTM step that carries GNN edge tiles


def _tbucket(t):
    f = (t.astype(np.float32) / np.float32(MAX_TIME)) * np.float32(TIME_NUM)
    return np.clip(f.astype(np.int32), 0, TIME_NUM - 1)


def _wrap16(idx, total):
    """int16 index layout for dma_gather: idx i -> [i%16, i//16], replicated
    to all 8 GPSIMD 16-partition groups (each Q7 core reads its own block)."""
    a = np.zeros(total, np.int16)
    a[:len(idx)] = idx.astype(np.int16)
    w = a.reshape(-1, 16).T                       # (16, total/16)
    return np.tile(w, (8, 1))                     # (128, total/16)


def kernel(**inputs):
    global LAST_EXEC_NS, SIM_NS
    import concourse.bass as bass
    import concourse.tile as tile
    from concourse import bacc, mybir
    from concourse.bass_utils import run_bass_kernel_spmd

    f32 = np.float32
    dt = mybir.dt
    AF = mybir.ActivationFunctionType
    OP = mybir.AluOpType

    ch = np.asarray(inputs['cas_history']).astype(np.int64)       # (B,S)
    ct = np.asarray(inputs['cas_times']).astype(f32)              # (B,S)
    lengths = np.maximum(np.asarray(inputs['lengths']).astype(np.int64), 1)
    static_emb = np.asarray(inputs['static_emb']).astype(f32)     # (1M,E)
    time_emb = np.asarray(inputs['time_emb']).astype(f32)         # (50,E)
    pos_emb = np.asarray(inputs['pos_emb']).astype(f32)           # (100,E)
    W_ih = np.asarray(inputs['W_ih']).astype(f32)                 # (4E,E)
    W_hh = np.asarray(inputs['W_hh']).astype(f32)
    b_ih = np.asarray(inputs['b_ih']).astype(f32)
    b_hh = np.asarray(inputs['b_hh']).astype(f32)
    W_trans = np.asarray(inputs['W_trans']).astype(f32)           # (E,3E)
    b_trans = np.asarray(inputs['b_trans']).astype(f32)

    # gate order i,f,g,o -> i,f,o,g; double the g-gate pre-activation so that
    # tanh(z) = 2*sigmoid(2z)-1 lets all four gates share one sigmoid call.
    def perm(w):
        return np.concatenate([w[0:E], w[E:2 * E], w[3 * E:4 * E], w[2 * E:3 * E]], axis=0)

    W_ih_p, W_hh_p = perm(W_ih), perm(W_hh)
    b_g = perm((b_ih + b_hh)[:, None])[:, 0]                      # (4E,)
    TW_ih = time_emb @ W_ih_p.T + b_g[None, :]                    # (50,4E)
    PW = pos_emb @ W_ih_p.T                                       # (100,4E)
    W_ihT = W_ih_p.T.copy()                                       # (E,4E)
    W_hhT = W_hh_p.T.copy()
    for arr in (W_ihT, W_hhT, TW_ih, PW):
        arr[:, 3 * E:] *= 2.0

    sides = {}
    for side in ('root', 'leaf'):
        node_id = np.asarray(inputs[f'node_id_{side}']).astype(np.int64)
        esrc = np.asarray(inputs[f'edge_src_{side}']).astype(np.int64)
        edst = np.asarray(inputs[f'edge_dst_{side}']).astype(np.int64)
        etime = np.asarray(inputs[f'edge_time_{side}']).astype(f32)
        Wm = np.asarray(inputs[f'W_msg_{side}']).astype(f32)      # (E,2E)
        bm = np.asarray(inputs[f'b_msg_{side}']).astype(f32)
        sides[side] = dict(
            src_uid=node_id[esrc],
            tb=_tbucket(etime),
            core=(edst // NPG) // BC,
            gl=(edst // NPG) % BC,
            Wm1T=np.ascontiguousarray(Wm[:, :E].T),               # (E,E)
            TW=time_emb @ Wm[:, E:].T + bm[None, :],              # (50,E)
        )

    # ---- per-core sharding on host ----
    # Cascades are sorted by length (desc) within each core so LSTM step t
    # only touches the still-alive prefix of width n_t (shared across cores).
    core_data = []
    max_edges = 0
    for c in range(NCORES):
        d = {}
        bs = slice(c * BC, (c + 1) * BC)
        lens_c = lengths[bs]
        order = np.argsort(-lens_c, kind='stable')
        d['order'] = order
        inv = np.empty(BC, np.int64)
        inv[order] = np.arange(BC)
        d['lens'] = lens_c[order]                                 # sorted desc
        d['cas_uid_sb'] = ch[bs][order]                           # (BC,S) sorted
        d['cas_tb_sb'] = _tbucket(ct[bs])[order]
        for side in ('root', 'leaf'):
            sd = sides[side]
            m = sd['core'] == c
            d[f'{side}_uid'] = sd['src_uid'][m]
            d[f'{side}_tb'] = sd['tb'][m]
            d[f'{side}_gl'] = inv[sd['gl'][m]]                    # remap to sorted
            max_edges = max(max_edges, int(m.sum()))
        core_data.append(d)

    # alive-width schedule: n_t = max over cores of #{len >= t+1}, mult of 4
    nts = []
    for t in range(S):
        m = max(int((d['lens'] >= t + 1).sum()) for d in core_data)
        nts.append(min(BC, max(4, (m + 3) // 4 * 4)))
    offs = np.concatenate([[0], np.cumsum(nts)]).astype(int)      # (S+1,)
    NS_P = ((int(offs[-1]) + P - 1) // P) * P                     # padded total

    NT_E = (max_edges + P - 1) // P                               # edge tiles/side
    CAPE = NT_E * P

    # pack cascade samples: step t contributes its alive prefix [0:n_t)
    NPACK = int(offs[-1])
    for d in core_data:
        uid_p = np.zeros(NPACK, np.int64)
        tb_p = np.zeros(NPACK, np.int32)
        for t in range(S):
            uid_p[offs[t]:offs[t + 1]] = d['cas_uid_sb'][0:nts[t], t]
            tb_p[offs[t]:offs[t + 1]] = d['cas_tb_sb'][0:nts[t], t]
        d['cas_uid_p'] = uid_p
        d['cas_tb_p'] = tb_p

    # compact per-core embedding table (shared by cascade + both edge sides)
    Umax = 0
    for d in core_data:
        all_uid = np.concatenate([d['cas_uid_p'], d['root_uid'], d['leaf_uid']])
        uniq, inv = np.unique(all_uid, return_inverse=True)
        n0, n1 = len(d['cas_uid_p']), len(d['root_uid'])
        d['cas_se'] = inv[:n0]
        d['root_se'] = inv[n0:n0 + n1]
        d['leaf_se'] = inv[n0 + n1:]
        d['uniq'] = uniq
        Umax = max(Umax, len(uniq))
    U = Umax
    assert U < 32768, f"compact table {U} rows exceeds int16 index range"

    def onehot50(tb, total):
        oh = np.zeros((TIME_NUM, total), BF16)
        oh[tb, np.arange(len(tb))] = 1.0
        return oh

    # pos one-hot (packed layout): sample s in [offs[t], offs[t+1]) -> row t
    oht_pos = np.zeros((NPG, NS_P), BF16)
    for t in range(S):
        oht_pos[t, offs[t]:offs[t + 1]] = 1.0

    in_maps = []
    for c, d in enumerate(core_data):
        tbl = np.zeros((U, E), BF16)
        tbl[:len(d['uniq'])] = static_emb[d['uniq']].astype(BF16)
        selm = np.zeros((NSAMP,), BF16)
        selm[(d['lens'] - 1) * BC + np.arange(BC)] = 1.0
        m = {
            'tbl': tbl,
            'idx_cas': _wrap16(d['cas_se'], NS_P),
            'oht_cas': onehot50(d['cas_tb_p'], NS_P),
            'oht_pos': oht_pos,
            'selmask': np.broadcast_to(selm, (P, NSAMP)).copy(),
            'W_ihT': W_ihT.astype(BF16),
            'W_hhT': W_hhT.astype(BF16),
            'TW_ih': TW_ih.astype(BF16),
            'PW': PW.astype(BF16),
            'identf': np.eye(P, dtype=f32),
            'Wt1T': np.ascontiguousarray(W_trans[:, :E].T).astype(BF16),
            'Wt2T': np.ascontiguousarray(W_trans[:, E:2 * E].T).astype(BF16),
            'Wt3T': np.ascontiguousarray(W_trans[:, 2 * E:].T).astype(BF16),
            'btr': b_trans[:, None].copy(),                       # (E,1) f32
        }
        for side in ('root', 'leaf'):
            s0 = side[0]
            ne = len(d[f'{side}_se'])
            oh = np.zeros((P, NT_E * BC), BF16)
            gl = d[f'{side}_gl']
            eidx = np.arange(ne)
            oh[eidx % P, (eidx // P) * BC + gl] = 1.0 / NPG
            m[f'idx_{s0}'] = _wrap16(d[f'{side}_se'], CAPE)
            m[f'oht_{s0}'] = onehot50(d[f'{side}_tb'], CAPE)
            m[f'oh_{s0}'] = oh
            m[f'TW_{s0}'] = sides[side]['TW'].astype(BF16)
            m[f'Wm1T_{s0}'] = sides[side]['Wm1T'].astype(BF16)
        in_maps.append(m)

    # ---- build the SPMD bass program (identical on all cores) ----
    nc = bacc.Bacc("TRN2", target_bir_lowering=False, debug=False,
                   enable_asserts=False, num_devices=NCORES)
    dr = {}
    for name, arr in in_maps[0].items():
        if arr.dtype == BF16:
            kd = dt.bfloat16
        elif arr.dtype == np.int16:
            kd = dt.int16
        else:
            kd = dt.float32
        dr[name] = nc.dram_tensor(name, list(arr.shape), kd, kind="ExternalInput")
    out_d = nc.dram_tensor("out", [E, BC], dt.float32, kind="ExternalOutput")

    NGNN = 2 * NT_E                                               # total edge tiles

    with tile.TileContext(nc) as tc:
        with (
            tc.tile_pool(name="const", bufs=1) as cp,
            tc.tile_pool(name="big", bufs=1) as bp,
            tc.tile_pool(name="work", bufs=3) as wp,
            tc.tile_pool(name="hold", bufs=1) as hp,
            tc.tile_pool(name="fifo", bufs=LOOKAHEAD, space="PSUM") as fifop,
            tc.tile_pool(name="msgp", bufs=3, space="PSUM") as msgp,
            tc.tile_pool(name="gaccp", bufs=1, space="PSUM") as gaccp,
        ):
            def load_const(name, dtyp=dt.float32):
                arr = in_maps[0][name]
                t = cp.tile(list(arr.shape), dtyp, tag=name, name=name)
                nc.sync.dma_start(t[:], dr[name][:])
                return t

            # load order = HWDGE queue order = DMA priority. Everything the
            # first LSTM steps need comes first; edge-side index tiles come
            # LAST so the big edge gathers queue behind the critical consts.
            idx_t = {'idx_cas': load_const('idx_cas', dt.int16)}
            # one-hot tables loaded in two chunks: the early columns unblock
            # the first LSTM steps, the rest follows the cascade gather.
            arr = in_maps[0]['oht_cas']
            oht_cas_t = cp.tile(list(arr.shape), dt.bfloat16, tag='oht_cas',
                                name='oht_cas')
            arr = in_maps[0]['oht_pos']
            oht_pos_t = cp.tile(list(arr.shape), dt.bfloat16, tag='oht_pos',
                                name='oht_pos')
            OSPL = 1536
            nc.sync.dma_start(oht_cas_t[:, 0:OSPL], dr['oht_cas'][:, 0:OSPL])
            nc.sync.dma_start(oht_pos_t[:, 0:OSPL], dr['oht_pos'][:, 0:OSPL])
            W_ihT_t = load_const('W_ihT', dt.bfloat16)
            W_hhT_t = load_const('W_hhT', dt.bfloat16)
            TW_ih_t = load_const('TW_ih', dt.bfloat16)
            PW_t = load_const('PW', dt.bfloat16)

            # cascade gather first: small first chunk so step 0 starts early
            xt_cas = bp.tile([P, 1, NS_P], dt.bfloat16, tag="xt_cas")
            CH0 = min(1536, NS_P - P) if NS_P > 2 * P else 0
            if CH0 > 0:
                nc.gpsimd.dma_gather(xt_cas[:, :, 0:CH0], dr['tbl'][:],
                                     idx_t['idx_cas'][:, 0:CH0 // 16], CH0, CH0,
                                     E, transpose=True, single_packet=False)
            nc.gpsimd.dma_gather(xt_cas[:, :, CH0:NS_P], dr['tbl'][:],
                                 idx_t['idx_cas'][:, CH0 // 16:NS_P // 16],
                                 NS_P - CH0, NS_P - CH0, E, transpose=True,
                                 single_packet=False)

            nc.sync.dma_start(oht_cas_t[:, OSPL:], dr['oht_cas'][:, OSPL:])
            nc.sync.dma_start(oht_pos_t[:, OSPL:], dr['oht_pos'][:, OSPL:])
            oht_e, oh_e, TW_e, Wm1T_e = {}, {}, {}, {}
            for s0 in ('r', 'l'):
                oht_e[s0] = load_const(f'oht_{s0}', dt.bfloat16)
                oh_e[s0] = load_const(f'oh_{s0}', dt.bfloat16)
                TW_e[s0] = load_const(f'TW_{s0}', dt.bfloat16)
                Wm1T_e[s0] = load_const(f'Wm1T_{s0}', dt.bfloat16)
            selm_t = load_const('selmask', dt.bfloat16)
            ident_t = load_const('identf')
            btr_t = load_const('btr')
            Wt_t = [load_const(n, dt.bfloat16) for n in ('Wt1T', 'Wt2T', 'Wt3T')]
            for n in ('idx_r', 'idx_l'):
                idx_t[n] = load_const(n, dt.int16)

            xt_e = {}
            for s0 in ('r', 'l'):
                xt_e[s0] = bp.tile([P, 1, CAPE], dt.bfloat16, tag=f"xt_{s0}",
                                   name=f"xt_{s0}")
                nc.gpsimd.dma_gather(xt_e[s0][:, :, :], dr['tbl'][:],
                                     idx_t[f'idx_{s0}'][:], CAPE, CAPE, E,
                                     transpose=True, single_packet=False)

            # ---- persistent state ----
            hbuf = bp.tile([P, S, BC], dt.bfloat16, tag="hbuf")
            nc.vector.memset(hbuf[:], 0.0)     # dead columns must read as 0
            hzero = hp.tile([P, BC], dt.bfloat16, tag="hzero")
            nc.vector.memset(hzero[:], 0.0)
            c_st = hp.tile([P, BC], dt.float32, tag="c_st")
            nc.vector.memset(c_st[:], 0.0)
            gacc = {}
            for s0 in ('r', 'l'):
                gacc[s0] = gaccp.tile([BC, E], dt.float32, tag=f"gacc_{s0}",
                                      name=f"gacc_{s0}")

            # ---- gate-FIFO precompute: one psum BANK per step ----
            # PSUM accumulation groups are per 2KB zero-region and reads
            # require a closed group, so each step's gates get their own bank:
            # 12 input-contribution matmuls run ahead (group opens, zeroing
            # the bank), 4 recurrent matmuls accumulate at step time (last
            # one closes the group), then the activation reads it.
            fifo_tiles = {}

            def emit_precompute(t):
                ps = fifop.tile([P, 4, P], dt.float32, tag="fifo", name="ps")
                c0, nt = int(offs[t]), nts[t]
                for k in range(4):
                    ks = slice(k * E, (k + 1) * E)
                    nc.tensor.matmul(out=ps[:, k, 0:nt], lhsT=W_ihT_t[:, ks],
                                     rhs=xt_cas[:, 0, c0:c0 + nt],
                                     start=(k == 0), stop=False)
                    nc.tensor.matmul(out=ps[:, k, 0:nt], lhsT=TW_ih_t[0:TIME_NUM, ks],
                                     rhs=oht_cas_t[0:TIME_NUM, c0:c0 + nt],
                                     start=False, stop=False)
                    nc.tensor.matmul(out=ps[:, k, 0:nt], lhsT=PW_t[0:NPG, ks],
                                     rhs=oht_pos_t[0:NPG, c0:c0 + nt],
                                     start=False, stop=False)
                fifo_tiles[t] = ps

            # ---- GNN edge tile ----
            gnn_seq = [(s0, i) for s0 in ('r', 'l') for i in range(NT_E)]
            gacc_started = {'r': False, 'l': False}
            relu_q = []
            scatter_q = []

            def emit_gnn_msg(s0, i):
                mp = msgp.tile([P, E], dt.float32, tag="msg", name="mp")
                es = slice(i * P, (i + 1) * P)
                nc.tensor.matmul(out=mp[:], lhsT=xt_e[s0][:, 0, es],
                                 rhs=Wm1T_e[s0][:], start=True, stop=False)
                nc.tensor.matmul(out=mp[:], lhsT=oht_e[s0][0:TIME_NUM, es],
                                 rhs=TW_e[s0][0:TIME_NUM, :], start=False, stop=True)
                relu_q.append((s0, i, mp))

            def emit_gnn_relu():
                if not relu_q:
                    return
                s0, i, mp = relu_q.pop(0)
                mr = wp.tile([P, E], dt.bfloat16, tag="mr", name="mr")
                nc.vector.tensor_scalar_max(mr[:], mp[:], 0.0)
                scatter_q.append((s0, i, mr))

            def emit_gnn_scatter():
                if not scatter_q:
                    return
                s0, i, mr = scatter_q.pop(0)
                nc.tensor.matmul(out=gacc[s0][:], lhsT=oh_e[s0][:, i * BC:(i + 1) * BC],
                                 rhs=mr[:], start=(i == 0), stop=(i == NT_E - 1))

            for t0 in range(LOOKAHEAD):
                emit_precompute(t0)

            # ---- LSTM over S steps ----
            gnn_ptr = 0
            hf_acc = None
            for t in range(S):
                if t + LOOKAHEAD < S:
                    emit_precompute(t + LOOKAHEAD)
                ps = fifo_tiles.pop(t)
                nt = nts[t]
                h_prev = hzero[:, 0:nt] if t == 0 else hbuf[:, t - 1, 0:nt]
                for k in (3, 0, 1, 2):
                    nc.tensor.matmul(out=ps[:, k, 0:nt],
                                     lhsT=W_hhT_t[:, k * E:(k + 1) * E],
                                     rhs=h_prev, start=False,
                                     stop=(k == 2))
                # GNN matmuls ride the PE queue after the recurrent matmuls
                do_gnn = GNN_T0 <= t and gnn_ptr < NGNN
                nmsg = 2 if do_gnn else 0
                if do_gnn:
                    for _ in range(nmsg):
                        if gnn_ptr < NGNN:
                            emit_gnn_msg(*gnn_seq[gnn_ptr])
                            gnn_ptr += 1
                    for _ in range(nmsg):
                        emit_gnn_scatter()
                sig = wp.tile([P, 4, BC], dt.bfloat16, tag="sig", name="sig")
                nc.scalar.activation(sig[:, :, 0:nt], ps[:, :, 0:nt], AF.Sigmoid)
                # c = sf*c + si*(2*sg - 1), fused as:
                #   t1 = (sg - 0.5) * si ;  c = (t1 * 2) + (c * sf)
                t1 = wp.tile([P, BC], dt.bfloat16, tag="t1", name="t1")
                nc.vector.scalar_tensor_tensor(
                    out=t1[:, 0:nt], in0=sig[:, 3, 0:nt], scalar=0.5,
                    in1=sig[:, 0, 0:nt], op0=OP.subtract, op1=OP.mult)
                nc.vector.tensor_tensor(out=c_st[:, 0:nt], in0=c_st[:, 0:nt],
                                        in1=sig[:, 1, 0:nt], op=OP.mult)
                nc.vector.scalar_tensor_tensor(
                    out=c_st[:, 0:nt], in0=t1[:, 0:nt], scalar=2.0,
                    in1=c_st[:, 0:nt], op0=OP.mult, op1=OP.add)
                th = wp.tile([P, BC], dt.bfloat16, tag="th", name="th")
                nc.scalar.activation(th[:, 0:nt], c_st[:, 0:nt], AF.Tanh)
                for _ in range(nmsg):
                    emit_gnn_relu()
                nc.vector.tensor_tensor(out=hbuf[:, t, 0:nt], in0=sig[:, 2, 0:nt],
                                        in1=th[:, 0:nt], op=OP.mult)
                # h-selection trickled in small chunks (10 steps each) so the
                # scheduler can't park a big reduce in the step chain.
                HCH = 10
                if t >= HCH + 2 and t % HCH == 2:
                    k = t // HCH - 1               # steps [k*HCH,(k+1)*HCH)
                    s0_, s1_ = k * HCH, (k + 1) * HCH
                    nc.vector.tensor_tensor(
                        out=hbuf[:, s0_:s1_, :], in0=hbuf[:, s0_:s1_, :],
                        in1=selm_t[:, s0_ * BC:s1_ * BC], op=OP.mult)
                if t >= HCH + 7 and t % HCH == 7:
                    k = t // HCH - 1
                    s0_, s1_ = k * HCH, (k + 1) * HCH
                    hp_k = wp.tile([P, BC], dt.float32, tag="hfp", name="hfp")
                    nc.vector.tensor_reduce(
                        out=hp_k[:], in_=hbuf[:, s0_:s1_, :].transpose([0, 2, 1]),
                        axis=mybir.AxisListType.X, op=OP.add)
                    if hf_acc is None:
                        hf_acc = hp.tile([P, BC], dt.float32, tag="hfacc")
                        nc.vector.tensor_copy(out=hf_acc[:], in_=hp_k[:])
                    else:
                        nc.vector.tensor_tensor(out=hf_acc[:], in0=hf_acc[:],
                                                in1=hp_k[:], op=OP.add)

            # drain any remaining GNN work
            while relu_q:
                emit_gnn_relu()
            while scatter_q:
                emit_gnn_scatter()

            # ---- graph embeddings: copy, transpose, cast (gacc done ~t=72) ----
            gT = {}
            for s0 in ('r', 'l'):
                gsb = hp.tile([BC, E], dt.float32, tag=f"gsb_{s0}", name=f"gsb_{s0}")
                nc.vector.tensor_copy(out=gsb[:], in_=gacc[s0][:])
                tp = msgp.tile([P, E], dt.float32, tag="msg", name="tp")
                nc.tensor.transpose(out=tp[:, 0:BC], in_=gsb[:],
                                    identity=ident_t[0:BC, 0:BC])
                g_t = hp.tile([P, BC], dt.bfloat16, tag=f"gT_{s0}", name=f"gT_{s0}")
                nc.vector.tensor_copy(out=g_t[:], in_=tp[:, 0:BC])
                gT[s0] = g_t

            # ---- final h extraction: last chunk + combine ----
            done = 9 * HCH                        # chunks 0..8 handled in-loop
            nc.vector.tensor_tensor(out=hbuf[:, done:, :], in0=hbuf[:, done:, :],
                                    in1=selm_t[:, done * BC:], op=OP.mult)
            hf = hp.tile([P, BC], dt.float32, tag="hf")
            nc.vector.tensor_reduce(
                out=hf[:], in_=hbuf[:, done:, :].transpose([0, 2, 1]),
                axis=mybir.AxisListType.X, op=OP.add)
            nc.vector.tensor_tensor(out=hf[:], in0=hf[:], in1=hf_acc[:],
                                    op=OP.add)
            hfb = hp.tile([P, BC], dt.bfloat16, tag="hfb")
            nc.vector.tensor_copy(out=hfb[:], in_=hf[:])

            # ---- final linear (transposed) + relu ----
            ops = msgp.tile([P, E], dt.float32, tag="msg", name="ops")
            for k, rhs_t in enumerate((hfb, gT['r'], gT['l'])):
                nc.tensor.matmul(out=ops[:, 0:BC], lhsT=Wt_t[k][:], rhs=rhs_t[:],
                                 start=(k == 0), stop=(k == 2))
            res = hp.tile([P, BC], dt.float32, tag="res")
            nc.scalar.activation(res[:], ops[:, 0:BC], AF.Relu, bias=btr_t[:, 0:1])
            nc.sync.dma_start(out_d[:], res[:])

    nc.compile()
    if SIMULATE:
        from trails.perfetto import LazyPerfetto
        for meth in ('enable_explicit_ordering', 'reserve_process_order'):
            if not hasattr(LazyPerfetto, meth):
                setattr(LazyPerfetto, meth, lambda self, *a, **k: None)
        from concourse.timeline_sim import TimelineSim
        ts = TimelineSim(nc, trace=bool(SIM_TRACE_PATH))
        SIM_NS = ts.simulate()
        if SIM_TRACE_PATH and ts.perfetto is not None:
            try:
                ts.perfetto.save(SIM_TRACE_PATH)
            except Exception:
                pass
    if SKIP_RUN:
        return np.zeros((B, E), np.float32)
    r = run_bass_kernel_spmd(nc, in_maps, core_ids=list(range(NCORES)),
                             trace=TRACE)
    LAST_EXEC_NS = r.exec_time_ns
    out = np.zeros((B, E), np.float32)
    for c in range(NCORES):
        res = np.asarray(r.results[c]["out"]).T.astype(np.float32)  # sorted order
        out[c * BC + core_data[c]['order']] = res
    return out


# revision 84
# speedup vs baseline: 1.0914x; 1.0914x over previous
"""AggregateEmbedding Trainium2 kernel: 8-core SPMD Bass/Tile implementation.

Sharding: data-parallel over cascades/graphs (64 per core). Edges are routed
host-side to the core that owns their destination graph. The 1M-row embedding
table is compacted per core to its touched rows (ids remapped to int16) and
stored bf16 in DRAM; the kernel performs three large transposed dma_gather
calls (cascade / root / leaf), so the random-row gather happens on device with
one ~1us SWDGE fixed cost per call instead of per 128 rows.

Structure on device (per core):
  - xT gathers land as [E=128, n] bf16, directly in matmul-operand layout.
  - Cascades are sorted by length (desc); LSTM step t only processes the
    alive prefix of width n_t (max over cores), and the cascade gather is
    packed accordingly.
  - The LSTM runs fully transposed ([E, batch] tiles, no per-step transpose).
    Each step owns one PSUM bank: 12 input-contribution matmuls (x @ W_ih,
    time-onehot @ TW_ih, pos-onehot @ PW) run a few steps ahead; 4 recurrent
    matmuls (W_hh_k @ h) accumulate at step time and close the bank's
    accumulation group. No xw SBUF roundtrip.
  - All 4 gates use sigmoid (tanh(z) = 2*sigmoid(2z)-1 with the g-gate
    weights pre-doubled host-side). The activation is split sigma_ifg /
    sigma_o so the critical path only waits for the three gates the cell
    update needs; the c-update is two fused scalar_tensor_tensor ops.
  - h history is written to SBUF bf16; the final h per cascade is extracted
    via a selection mask + strided reduces trickled through the loop.
  - Tree GNN message passing (both sides) runs interleaved with LSTM steps:
    per 128-edge tile, 2 matmuls (x_src @ W1 + time-onehot @ TW) -> relu on
    DVE -> scatter matmul with a host-built one-hot (scaled by 1/100 for the
    mean) accumulating into a per-side PSUM tile.
  - Final: out^T = relu(sum_k WtT_k^T @ featT_k + b) computed transposed so
    the bias is a per-partition scalar; host transposes back and un-permutes.
"""

import numpy as np
import ml_dtypes

BF16 = ml_dtypes.bfloat16

E = 128
S = 100
B = 512
NCORES = 8
BC = B // NCORES          # cascades / graphs per core
TIME_NUM = 50
MAX_TIME = 1.0
NPG = 100                 # nodes per graph
P = 128
NSAMP = S * BC            # 6400 cascade samples per core
NBLK = NSAMP // P         # 50 two-step blocks

LAST_EXEC_NS = None
TRACE = False
SIMULATE = False            # run TimelineSim cost model after compile
SIM_NS = None
SIM_TRACE_PATH = "/tmp/timeline.pftrace"
SKIP_RUN = False            # skip HW execution (sim-only iteration)

LOOKAHEAD = 2               # gate-FIFO psum blocks in flight
GNN_T0 = 20                 # first LSTM step that carries GNN edge tiles


def _tbucket(t):
    f = (t.astype(np.float32) / np.float32(MAX_TIME)) * np.float32(TIME_NUM)
    return np.clip(f.astype(np.int32), 0, TIME_NUM - 1)


def _wrap16(idx, total):
    """int16 index layout for dma_gather: idx i -> [i%16, i//16], replicated
    to all 8 GPSIMD 16-partition groups (each Q7 core reads its own block)."""
    a = np.zeros(total, np.int16)
    a[:len(idx)] = idx.astype(np.int16)
    w = a.reshape(-1, 16).T                       # (16, total/16)
    return np.tile(w, (8, 1))                     # (128, total/16)


def kernel(**inputs):
    global LAST_EXEC_NS, SIM_NS
    import concourse.bass as bass
    import concourse.tile as tile
    from concourse import bacc, mybir
    from concourse.bass_utils import run_bass_kernel_spmd

    f32 = np.float32
    dt = mybir.dt
    AF = mybir.ActivationFunctionType
    OP = mybir.AluOpType

    ch = np.asarray(inputs['cas_history']).astype(np.int64)       # (B,S)
    ct = np.asarray(inputs['cas_times']).astype(f32)              # (B,S)
    lengths = np.maximum(np.asarray(inputs['lengths']).astype(np.int64), 1)
    static_emb = np.asarray(inputs['static_emb']).astype(f32)     # (1M,E)
    time_emb = np.asarray(inputs['time_emb']).astype(f32)         # (50,E)
    pos_emb = np.asarray(inputs['pos_emb']).astype(f32)           # (100,E)
    W_ih = np.asarray(inputs['W_ih']).astype(f32)                 # (4E,E)
    W_hh = np.asarray(inputs['W_hh']).astype(f32)
    b_ih = np.asarray(inputs['b_ih']).astype(f32)
    b_hh = np.asarray(inputs['b_hh']).astype(f32)
    W_trans = np.asarray(inputs['W_trans']).astype(f32)           # (E,3E)
    b_trans = np.asarray(inputs['b_trans']).astype(f32)

    # gate order stays i,f,g,o; double the g-gate pre-activation so that
    # tanh(z) = 2*sigmoid(2z)-1 lets all gates use sigmoid.
    def perm(w):
        return w.copy()

    W_ih_p, W_hh_p = perm(W_ih), perm(W_hh)
    b_g = perm((b_ih + b_hh)[:, None])[:, 0]                      # (4E,)
    TW_ih = time_emb @ W_ih_p.T + b_g[None, :]                    # (50,4E)
    PW = pos_emb @ W_ih_p.T                                       # (100,4E)
    W_ihT = W_ih_p.T.copy()                                       # (E,4E)
    W_hhT = W_hh_p.T.copy()
    for arr in (W_ihT, W_hhT, TW_ih, PW):
        arr[:, 2 * E:3 * E] *= 2.0

    sides = {}
    for side in ('root', 'leaf'):
        node_id = np.asarray(inputs[f'node_id_{side}']).astype(np.int64)
        esrc = np.asarray(inputs[f'edge_src_{side}']).astype(np.int64)
        edst = np.asarray(inputs[f'edge_dst_{side}']).astype(np.int64)
        etime = np.asarray(inputs[f'edge_time_{side}']).astype(f32)
        Wm = np.asarray(inputs[f'W_msg_{side}']).astype(f32)      # (E,2E)
        bm = np.asarray(inputs[f'b_msg_{side}']).astype(f32)
        sides[side] = dict(
            src_uid=node_id[esrc],
            tb=_tbucket(etime),
            core=(edst // NPG) // BC,
            gl=(edst // NPG) % BC,
            Wm1T=np.ascontiguousarray(Wm[:, :E].T),               # (E,E)
            TW=time_emb @ Wm[:, E:].T + bm[None, :],              # (50,E)
        )

    # ---- per-core sharding on host ----
    # Cascades are sorted by length (desc) within each core so LSTM step t
    # only touches the still-alive prefix of width n_t (shared across cores).
    core_data = []
    max_edges = 0
    for c in range(NCORES):
        d = {}
        bs = slice(c * BC, (c + 1) * BC)
        lens_c = lengths[bs]
        order = np.argsort(-lens_c, kind='stable')
        d['order'] = order
        inv = np.empty(BC, np.int64)
        inv[order] = np.arange(BC)
        d['lens'] = lens_c[order]                                 # sorted desc
        d['cas_uid_sb'] = ch[bs][order]                           # (BC,S) sorted
        d['cas_tb_sb'] = _tbucket(ct[bs])[order]
        for side in ('root', 'leaf'):
            sd = sides[side]
            m = sd['core'] == c
            d[f'{side}_uid'] = sd['src_uid'][m]
            d[f'{side}_tb'] = sd['tb'][m]
            d[f'{side}_gl'] = inv[sd['gl'][m]]                    # remap to sorted
            max_edges = max(max_edges, int(m.sum()))
        core_data.append(d)

    # alive-width schedule: n_t = max over cores of #{len >= t+1}, mult of 4
    nts = []
    for t in range(S):
        m = max(int((d['lens'] >= t + 1).sum()) for d in core_data)
        nts.append(min(BC, max(4, (m + 3) // 4 * 4)))
    offs = np.concatenate([[0], np.cumsum(nts)]).astype(int)      # (S+1,)
    NS_P = ((int(offs[-1]) + P - 1) // P) * P                     # padded total

    NT_E = (max_edges + P - 1) // P                               # edge tiles/side
    CAPE = NT_E * P

    # pack cascade samples: step t contributes its alive prefix [0:n_t)
    NPACK = int(offs[-1])
    for d in core_data:
        uid_p = np.zeros(NPACK, np.int64)
        tb_p = np.zeros(NPACK, np.int32)
        for t in range(S):
            uid_p[offs[t]:offs[t + 1]] = d['cas_uid_sb'][0:nts[t], t]
            tb_p[offs[t]:offs[t + 1]] = d['cas_tb_sb'][0:nts[t], t]
        d['cas_uid_p'] = uid_p
        d['cas_tb_p'] = tb_p

    # compact per-core embedding table (shared by cascade + both edge sides)
    Umax = 0
    for d in core_data:
        all_uid = np.concatenate([d['cas_uid_p'], d['root_uid'], d['leaf_uid']])
        uniq, inv = np.unique(all_uid, return_inverse=True)
        n0, n1 = len(d['cas_uid_p']), len(d['root_uid'])
        d['cas_se'] = inv[:n0]
        d['root_se'] = inv[n0:n0 + n1]
        d['leaf_se'] = inv[n0 + n1:]
        d['uniq'] = uniq
        Umax = max(Umax, len(uniq))
    U = Umax
    assert U < 32768, f"compact table {U} rows exceeds int16 index range"

    def onehot50(tb, total):
        oh = np.zeros((TIME_NUM, total), BF16)
        oh[tb, np.arange(len(tb))] = 1.0
        return oh

    # pos one-hot (packed layout): sample s in [offs[t], offs[t+1]) -> row t
    oht_pos = np.zeros((NPG, NS_P), BF16)
    for t in range(S):
        oht_pos[t, offs[t]:offs[t + 1]] = 1.0

    in_maps = []
    for c, d in enumerate(core_data):
        tbl = np.zeros((U, E), BF16)
        tbl[:len(d['uniq'])] = static_emb[d['uniq']].astype(BF16)
        selm = np.zeros((NSAMP,), BF16)
        selm[(d['lens'] - 1) * BC + np.arange(BC)] = 1.0
        m = {
            'tbl': tbl,
            'idx_cas': _wrap16(d['cas_se'], NS_P),
            'oht_cas': onehot50(d['cas_tb_p'], NS_P),
            'oht_pos': oht_pos,
            'selmask': np.broadcast_to(selm, (P, NSAMP)).copy(),
            'W_ihT': W_ihT.astype(BF16),
            'W_hhT': W_hhT.astype(BF16),
            'TW_ih': TW_ih.astype(BF16),
            'PW': PW.astype(BF16),
            'identf': np.eye(P, dtype=f32),
            'Wt1T': np.ascontiguousarray(W_trans[:, :E].T).astype(BF16),
            'Wt2T': np.ascontiguousarray(W_trans[:, E:2 * E].T).astype(BF16),
            'Wt3T': np.ascontiguousarray(W_trans[:, 2 * E:].T).astype(BF16),
            'btr': b_trans[:, None].copy(),                       # (E,1) f32
        }
        for side in ('root', 'leaf'):
            s0 = side[0]
            ne = len(d[f'{side}_se'])
            oh = np.zeros((P, NT_E * BC), BF16)
            gl = d[f'{side}_gl']
            eidx = np.arange(ne)
            oh[eidx % P, (eidx // P) * BC + gl] = 1.0 / NPG
            m[f'idx_{s0}'] = _wrap16(d[f'{side}_se'], CAPE)
            m[f'oht_{s0}'] = onehot50(d[f'{side}_tb'], CAPE)
            m[f'oh_{s0}'] = oh
            m[f'TW_{s0}'] = sides[side]['TW'].astype(BF16)
            m[f'Wm1T_{s0}'] = sides[side]['Wm1T'].astype(BF16)
        in_maps.append(m)

    # ---- build the SPMD bass program (identical on all cores) ----
    nc = bacc.Bacc("TRN2", target_bir_lowering=False, debug=False,
                   enable_asserts=False, num_devices=NCORES)
    dr = {}
    for name, arr in in_maps[0].items():
        if arr.dtype == BF16:
            kd = dt.bfloat16
        elif arr.dtype == np.int16:
            kd = dt.int16
        else:
            kd = dt.float32
        dr[name] = nc.dram_tensor(name, list(arr.shape), kd, kind="ExternalInput")
    out_d = nc.dram_tensor("out", [E, BC], dt.float32, kind="ExternalOutput")

    NGNN = 2 * NT_E                                               # total edge tiles

    with tile.TileContext(nc) as tc:
        with (
            tc.tile_pool(name="const", bufs=1) as cp,
            tc.tile_pool(name="big", bufs=1) as bp,
            tc.tile_pool(name="work", bufs=3) as wp,
            tc.tile_pool(name="hold", bufs=1) as hp,
            tc.tile_pool(name="fifo", bufs=LOOKAHEAD, space="PSUM") as fifop,
            tc.tile_pool(name="msgp", bufs=3, space="PSUM") as msgp,
            tc.tile_pool(name="gaccp", bufs=1, space="PSUM") as gaccp,
        ):
            def load_const(name, dtyp=dt.float32):
                arr = in_maps[0][name]
                t = cp.tile(list(arr.shape), dtyp, tag=name, name=name)
                nc.sync.dma_start(t[:], dr[name][:])
                return t

            # load order = HWDGE queue order = DMA priority. Everything the
            # first LSTM steps need comes first; edge-side index tiles come
            # LAST so the big edge gathers queue behind the critical consts.
            idx_t = {'idx_cas': load_const('idx_cas', dt.int16)}
            # one-hot tables loaded in two chunks: the early columns unblock
            # the first LSTM steps, the rest follows the cascade gather.
            arr = in_maps[0]['oht_cas']
            oht_cas_t = cp.tile(list(arr.shape), dt.bfloat16, tag='oht_cas',
                                name='oht_cas')
            arr = in_maps[0]['oht_pos']
            oht_pos_t = cp.tile(list(arr.shape), dt.bfloat16, tag='oht_pos',
                                name='oht_pos')
            OSPL = 1536
            nc.sync.dma_start(oht_cas_t[:, 0:OSPL], dr['oht_cas'][:, 0:OSPL])
            nc.sync.dma_start(oht_pos_t[:, 0:OSPL], dr['oht_pos'][:, 0:OSPL])
            W_ihT_t = load_const('W_ihT', dt.bfloat16)
            W_hhT_t = load_const('W_hhT', dt.bfloat16)
            TW_ih_t = load_const('TW_ih', dt.bfloat16)
            PW_t = load_const('PW', dt.bfloat16)

            # cascade gather first: small first chunk so step 0 starts early
            xt_cas = bp.tile([P, 1, NS_P], dt.bfloat16, tag="xt_cas")
            CH0 = min(1536, NS_P - P) if NS_P > 2 * P else 0
            if CH0 > 0:
                nc.gpsimd.dma_gather(xt_cas[:, :, 0:CH0], dr['tbl'][:],
                                     idx_t['idx_cas'][:, 0:CH0 // 16], CH0, CH0,
                                     E, transpose=True, single_packet=False)
            nc.gpsimd.dma_gather(xt_cas[:, :, CH0:NS_P], dr['tbl'][:],
                                 idx_t['idx_cas'][:, CH0 // 16:NS_P // 16],
                                 NS_P - CH0, NS_P - CH0, E, transpose=True,
                                 single_packet=False)

            nc.sync.dma_start(oht_cas_t[:, OSPL:], dr['oht_cas'][:, OSPL:])
            nc.sync.dma_start(oht_pos_t[:, OSPL:], dr['oht_pos'][:, OSPL:])
            oht_e, oh_e, TW_e, Wm1T_e = {}, {}, {}, {}
            for s0 in ('r', 'l'):
                oht_e[s0] = load_const(f'oht_{s0}', dt.bfloat16)
                oh_e[s0] = load_const(f'oh_{s0}', dt.bfloat16)
                TW_e[s0] = load_const(f'TW_{s0}', dt.bfloat16)
                Wm1T_e[s0] = load_const(f'Wm1T_{s0}', dt.bfloat16)
            selm_t = load_const('selmask', dt.bfloat16)
            ident_t = load_const('identf')
            btr_t = load_const('btr')
            Wt_t = [load_const(n, dt.bfloat16) for n in ('Wt1T', 'Wt2T', 'Wt3T')]
            for n in ('idx_r', 'idx_l'):
                idx_t[n] = load_const(n, dt.int16)

            xt_e = {}
            for s0 in ('r', 'l'):
                xt_e[s0] = bp.tile([P, 1, CAPE], dt.bfloat16, tag=f"xt_{s0}",
                                   name=f"xt_{s0}")
                nc.gpsimd.dma_gather(xt_e[s0][:, :, :], dr['tbl'][:],
                                     idx_t[f'idx_{s0}'][:], CAPE, CAPE, E,
                                     transpose=True, single_packet=False)

            # ---- persistent state ----
            hbuf = bp.tile([P, S, BC], dt.bfloat16, tag="hbuf")
            nc.vector.memset(hbuf[:], 0.0)     # dead columns must read as 0
            hzero = hp.tile([P, BC], dt.bfloat16, tag="hzero")
            nc.vector.memset(hzero[:], 0.0)
            c_st = hp.tile([P, BC], dt.float32, tag="c_st")
            nc.vector.memset(c_st[:], 0.0)
            gacc = {}
            for s0 in ('r', 'l'):
                gacc[s0] = gaccp.tile([BC, E], dt.float32, tag=f"gacc_{s0}",
                                      name=f"gacc_{s0}")

            # ---- gate-FIFO precompute: one psum BANK per step ----
            # PSUM accumulation groups are per 2KB zero-region and reads
            # require a closed group, so each step's gates get their own bank:
            # 12 input-contribution matmuls run ahead (group opens, zeroing
            # the bank), 4 recurrent matmuls accumulate at step time (last
            # one closes the group), then the activation reads it.
            fifo_tiles = {}

            def emit_precompute(t):
                ps = fifop.tile([P, 4, P], dt.float32, tag="fifo", name="ps")
                c0, nt = int(offs[t]), nts[t]
                for k in range(4):
                    ks = slice(k * E, (k + 1) * E)
                    nc.tensor.matmul(out=ps[:, k, 0:nt], lhsT=W_ihT_t[:, ks],
                                     rhs=xt_cas[:, 0, c0:c0 + nt],
                                     start=(k == 0), stop=False)
                    nc.tensor.matmul(out=ps[:, k, 0:nt], lhsT=TW_ih_t[0:TIME_NUM, ks],
                                     rhs=oht_cas_t[0:TIME_NUM, c0:c0 + nt],
                                     start=False, stop=False)
                    nc.tensor.matmul(out=ps[:, k, 0:nt], lhsT=PW_t[0:NPG, ks],
                                     rhs=oht_pos_t[0:NPG, c0:c0 + nt],
                                     start=False, stop=False)
                fifo_tiles[t] = ps

            # ---- GNN edge tile ----
            gnn_seq = [(s0, i) for s0 in ('r', 'l') for i in range(NT_E)]
            gacc_started = {'r': False, 'l': False}
            relu_q = []
            scatter_q = []

            def emit_gnn_msg(s0, i):
                mp = msgp.tile([P, E], dt.float32, tag="msg", name="mp")
                es = slice(i * P, (i + 1) * P)
                nc.tensor.matmul(out=mp[:], lhsT=xt_e[s0][:, 0, es],
                                 rhs=Wm1T_e[s0][:], start=True, stop=False)
                nc.tensor.matmul(out=mp[:], lhsT=oht_e[s0][0:TIME_NUM, es],
                                 rhs=TW_e[s0][0:TIME_NUM, :], start=False,
                                 stop=True)
                relu_q.append((s0, i, mp))

            def emit_gnn_relu():
                if not relu_q:
                    return
                s0, i, mp = relu_q.pop(0)
                mr = wp.tile([P, E], dt.bfloat16, tag="mr", name="mr")
                nc.vector.tensor_scalar_max(mr[:], mp[:], 0.0)
                scatter_q.append((s0, i, mr))

            def emit_gnn_scatter():
                if not scatter_q:
                    return
                s0, i, mr = scatter_q.pop(0)
                nc.tensor.matmul(out=gacc[s0][:],
                                 lhsT=oh_e[s0][:, i * BC:(i + 1) * BC],
                                 rhs=mr[:], start=(i == 0),
                                 stop=(i == NT_E - 1))

            for t0 in range(LOOKAHEAD):
                emit_precompute(t0)

            # ---- LSTM over S steps ----
            gnn_ptr = 0
            hf_acc = None
            for t in range(S):
                if t + LOOKAHEAD < S:
                    emit_precompute(t + LOOKAHEAD)
                ps = fifo_tiles.pop(t)
                nt = nts[t]
                h_prev = hzero[:, 0:nt] if t == 0 else hbuf[:, t - 1, 0:nt]
                with tc.high_priority(offset=200):
                    for k in (2, 0, 1, 3):
                        nc.tensor.matmul(out=ps[:, k, 0:nt],
                                         lhsT=W_hhT_t[:, k * E:(k + 1) * E],
                                         rhs=h_prev, start=False,
                                         stop=(k == 3))
                # GNN matmuls ride the PE queue after the recurrent matmuls
                do_gnn = GNN_T0 <= t and gnn_ptr < NGNN
                nmsg = 2 if do_gnn else 0
                if do_gnn:
                    for _ in range(nmsg):
                        if gnn_ptr < NGNN:
                            emit_gnn_msg(*gnn_seq[gnn_ptr])
                            gnn_ptr += 1
                    for _ in range(nmsg):
                        emit_gnn_scatter()
                sig = wp.tile([P, 4, BC], dt.bfloat16, tag="sig", name="sig")
                t1 = wp.tile([P, BC], dt.bfloat16, tag="t1", name="t1")
                th = wp.tile([P, BC], dt.bfloat16, tag="th", name="th")
                with tc.high_priority(offset=200):
                    nc.scalar.activation(sig[:, 0:3, 0:nt], ps[:, 0:3, 0:nt],
                                         AF.Sigmoid)
                    nc.scalar.activation(sig[:, 3, 0:nt], ps[:, 3, 0:nt],
                                         AF.Sigmoid)
                    # c = sf*c + si*(2*sg - 1), fused as:
                    #   t1 = (sg - 0.5) * si ;  c = (t1 * 2) + (c * sf)
                    nc.vector.scalar_tensor_tensor(
                        out=t1[:, 0:nt], in0=sig[:, 2, 0:nt], scalar=0.5,
                        in1=sig[:, 0, 0:nt], op0=OP.subtract, op1=OP.mult)
                    nc.vector.tensor_tensor(out=c_st[:, 0:nt], in0=c_st[:, 0:nt],
                                            in1=sig[:, 1, 0:nt], op=OP.mult)
                    nc.vector.scalar_tensor_tensor(
                        out=c_st[:, 0:nt], in0=t1[:, 0:nt], scalar=2.0,
                        in1=c_st[:, 0:nt], op0=OP.mult, op1=OP.add)
                    nc.scalar.activation(th[:, 0:nt], c_st[:, 0:nt], AF.Tanh)
                    nc.vector.tensor_tensor(out=hbuf[:, t, 0:nt],
                                            in0=sig[:, 3, 0:nt],
                                            in1=th[:, 0:nt], op=OP.mult)
                for _ in range(nmsg):
                    emit_gnn_relu()
                # h-selection trickled in small chunks (10 steps each) so the
                # scheduler can't park a big reduce in the step chain.
                HCH = 5
                if t >= HCH + 2 and (t - 2) % HCH == 0:
                    k = (t - 2) // HCH - 1         # steps [k*HCH,(k+1)*HCH)
                    s0_, s1_ = k * HCH, (k + 1) * HCH
                    with tc.high_priority(offset=-5000):
                        nc.vector.tensor_tensor(
                            out=hbuf[:, s0_:s1_, :], in0=hbuf[:, s0_:s1_, :],
                            in1=selm_t[:, s0_ * BC:s1_ * BC], op=OP.mult)
                if t >= HCH + 4 and (t - 4) % HCH == 0:
                    k = (t - 4) // HCH - 1
                    s0_, s1_ = k * HCH, (k + 1) * HCH
                    hp_k = wp.tile([P, BC], dt.float32, tag="hfp", name="hfp")
                    with tc.high_priority(offset=-5000):
                        nc.vector.tensor_reduce(
                            out=hp_k[:],
                            in_=hbuf[:, s0_:s1_, :].transpose([0, 2, 1]),
                            axis=mybir.AxisListType.X, op=OP.add)
                        if hf_acc is None:
                            hf_acc = hp.tile([P, BC], dt.float32, tag="hfacc")
                            nc.vector.tensor_copy(out=hf_acc[:], in_=hp_k[:])
                        else:
                            nc.vector.tensor_tensor(out=hf_acc[:], in0=hf_acc[:],
                                                    in1=hp_k[:], op=OP.add)

            # drain any remaining GNN work
            while relu_q:
                emit_gnn_relu()
            while scatter_q:
                emit_gnn_scatter()

            # ---- graph embeddings: copy, transpose, cast (gacc done ~t=72) ----
            gT = {}
            for s0 in ('r', 'l'):
                gsb = hp.tile([BC, E], dt.float32, tag=f"gsb_{s0}", name=f"gsb_{s0}")
                nc.vector.tensor_copy(out=gsb[:], in_=gacc[s0][:])
                tp = msgp.tile([P, E], dt.float32, tag="msg", name="tp")
                nc.tensor.transpose(out=tp[:, 0:BC], in_=gsb[:],
                                    identity=ident_t[0:BC, 0:BC])
                g_t = hp.tile([P, BC], dt.bfloat16, tag=f"gT_{s0}", name=f"gT_{s0}")
                nc.vector.tensor_copy(out=g_t[:], in_=tp[:, 0:BC])
                gT[s0] = g_t

            # ---- final h extraction: last chunk + combine ----
            done = 19 * HCH                        # chunks 0..8 handled in-loop
            nc.vector.tensor_tensor(out=hbuf[:, done:, :], in0=hbuf[:, done:, :],
                                    in1=selm_t[:, done * BC:], op=OP.mult)
            hf = hp.tile([P, BC], dt.float32, tag="hf")
            nc.vector.tensor_reduce(
                out=hf[:], in_=hbuf[:, done:, :].transpose([0, 2, 1]),
                axis=mybir.AxisListType.X, op=OP.add)
            nc.vector.tensor_tensor(out=hf[:], in0=hf[:], in1=hf_acc[:],
                                    op=OP.add)
            hfb = hp.tile([P, BC], dt.bfloat16, tag="hfb")
            nc.vector.tensor_copy(out=hfb[:], in_=hf[:])

            # ---- final linear (transposed) + relu ----
            ops = msgp.tile([P, E], dt.float32, tag="msg", name="ops")
            for k, rhs_t in enumerate((hfb, gT['r'], gT['l'])):
                nc.tensor.matmul(out=ops[:, 0:BC], lhsT=Wt_t[k][:], rhs=rhs_t[:],
                                 start=(k == 0), stop=(k == 3))
            res = hp.tile([P, BC], dt.float32, tag="res")
            nc.scalar.activation(res[:], ops[:, 0:BC], AF.Relu, bias=btr_t[:, 0:1])
            nc.sync.dma_start(out_d[:], res[:])

    nc.compile()
    if SIMULATE:
        from trails.perfetto import LazyPerfetto
        for meth in ('enable_explicit_ordering', 'reserve_process_order'):
            if not hasattr(LazyPerfetto, meth):
                setattr(LazyPerfetto, meth, lambda self, *a, **k: None)
        from concourse.timeline_sim import TimelineSim
        ts = TimelineSim(nc, trace=bool(SIM_TRACE_PATH))
        SIM_NS = ts.simulate()
        if SIM_TRACE_PATH and ts.perfetto is not None:
            try:
                ts.perfetto.save(SIM_TRACE_PATH)
            except Exception:
                pass
    if SKIP_RUN:
        return np.zeros((B, E), np.float32)
    r = run_bass_kernel_spmd(nc, in_maps, core_ids=list(range(NCORES)),
                             trace=TRACE)
    LAST_EXEC_NS = r.exec_time_ns
    out = np.zeros((B, E), np.float32)
    for c in range(NCORES):
        res = np.asarray(r.results[c]["out"]).T.astype(np.float32)  # sorted order
        out[c * BC + core_data[c]['order']] = res
    return out
